# revision 1
# baseline (speedup 1.0000x reference)
"""Distributed GQA attention prefill for TRN2 (8 NeuronCores).

Problem: T=2048, D=4096, N=32 query heads, K=8 kv heads, H=128.
    q = x @ w_q; k = x @ w_k; v = x @ w_v   (fused in the reference)
    rope(q), rope(k); causal GQA attention; out = o @ w_o

Sharding (tensor-parallel over heads): core c owns query heads
4c..4c+3 and kv head c (GQA groups align). w_q/w_o sharded on N,
w_k/w_v on K, x replicated. Each core computes its partial o_proj
output [T, D]; a chunked bf16 ReduceScatter sums partials; the host
concatenates the per-core row shards.

Single software-pipelined phase: the T=2048 sequence is processed in
4 panels of 512. Block b emits QKV+RoPE for panel b interleaved (in
PE program order) with attention+o_proj for panel b-1, so the PE
stream stays dense (TRN2's tensor engine drops to a 1.2GHz p-state
after any idle and only reaches 2.4GHz after 3us of continuous
execution). Output rows complete panel-by-panel, so the serial
ReduceScatter chain starts ~60us into the kernel and hides almost
entirely under compute; the last chunk is only 128 rows.

Attention per panel/head: S^T[s,t] = kT_sb.T @ qT_panel with the
diagonal blocks restricted to their causal column range, P^T =
exp(S^T/sqrt(H)) on ScalarE (scores are O(1); no max pass), causal
mask on the diagonal 128x128 tile via a DVE multiply, PV with a ones
column for row sums, DVE reciprocal+scale, PE transpose, o_proj
accumulating over heads into 2 rotating PSUM banks. PV transposes
are deferred past the next head's scores so the PE never sits on the
DVE normalize turnaround; the t-blocks run in order [1,2,3,0] so the
rows gating the final 1MB ReduceScatter chunk finish last-but-small;
the last panel's o_proj copy-outs all go to DVE (its block is
scalar-bound: the exp chain alone nearly fills ScalarE) with stores
on the then-idle sync queue.

Floors observed on HW: PE busy ~431us (the ~128-cycle LDWEIGHTS per
matmul is structural -- walrus ldw-opt rejects Bass's pre-split
InstLdweights); per-DMA-queue bandwidth ~100GB/s (3 queues); each RS
chunk internally moves ~3.7x its size through HBM (RDH), so the RS
chain is BW-coupled to compute. Measured best 550.5us, low-noise
band 553-566us vs 567-570us for the two-phase baseline (same-session
A/B); identical binaries drift to 587-647us as the device throttles
(variance appears inside PE-busy time), so cross-session timing
comparisons need matched thermal conditions. rel err 7.051e-3.
"""

import numpy as np
import ml_dtypes

T, D, NH, KH, H = 2048, 4096, 32, 8, 128
THETA = 10000.0
G = NH // KH          # 4 query heads per core
N_CORES = 8
TP_SIZE = 512         # t-panel
NTP = T // TP_SIZE    # 4 panels
NTB = T // 128        # 16 t/s blocks
NDB = D // 128        # 32 d blocks
NQ = 4                # x quarters per panel (8 d-blocks each)
QDB = NDB // NQ
SCALE = 1.0 / float(np.sqrt(H))
VEXT_STRIDE = 132     # v_ext row stride (129 used, padded)
# Output ReduceScatter chunks: (global_start_row, nrows). 128-row first
# chunk to warm the CC stream early, 128-row final chunk to shrink the
# serial tail.
RS_CHUNKS = [(0, 256), (256, 256), (512, 512), (1024, 384), (1408, 128),
             (1536, 128), (1664, 256), (1920, 128)]

_NC_CACHE = {}


def _enable_ldw_opt():
    """No-op: walrus's ldw-opt rejects Bass's pre-split InstLdweights
    ("InstLdweights is not compatible with LDW optimization"), so the
    ~128-cycle weight-load serialization per matmul is structural."""
    return




def _build_nc():
    import concourse.mybir as mybir
    import concourse.tile as tile
    from concourse import bacc
    from concourse.masks import make_identity

    BF16 = mybir.dt.bfloat16
    F32 = mybir.dt.float32
    EXP = mybir.ActivationFunctionType.Exp

    nc = bacc.Bacc("TRN2", target_bir_lowering=False, debug=False,
                   num_devices=N_CORES)

    xt_ext = nc.dram_tensor("xt", [128, NTP, NDB, TP_SIZE], BF16,
                            kind="ExternalInput")
    wq_ext = nc.dram_tensor("wq", [128, G, NDB, H], BF16,
                            kind="ExternalInput")
    wk_ext = nc.dram_tensor("wk", [128, NDB, H], BF16, kind="ExternalInput")
    wv_ext = nc.dram_tensor("wv", [128, NDB, H], BF16, kind="ExternalInput")
    wo_ext = nc.dram_tensor("wo", [128, G, D], BF16, kind="ExternalInput")
    cos_ext = nc.dram_tensor("cos_t", [H, T], BF16, kind="ExternalInput")
    sin_ext = nc.dram_tensor("sin_t", [H, T], BF16, kind="ExternalInput")
    mask_ext = nc.dram_tensor("maskp", [128, 128], BF16, kind="ExternalInput")
    out_ext = nc.dram_tensor("out", [T // N_CORES, D], BF16,
                             kind="ExternalOutput")

    with tile.TileContext(nc) as tc:
        with (
            tc.tile_pool(name="consts", bufs=1) as consts,
            tc.tile_pool(name="persist", bufs=1) as persist,
            tc.tile_pool(name="xqp", bufs=8) as xqp,
            tc.tile_pool(name="qtp", bufs=2) as qtp,
            tc.tile_pool(name="csp", bufs=2) as csp,
            tc.tile_pool(name="ptp", bufs=31) as ptp,
            tc.tile_pool(name="ropep", bufs=1) as ropep,
            tc.tile_pool(name="scp", bufs=4) as scp,
            tc.tile_pool(name="osbp", bufs=2) as osbp,
            tc.tile_pool(name="qkvps", bufs=1, space="PSUM") as qkv_ps,
            tc.tile_pool(name="sps", bufs=2, space="PSUM") as sps,
            tc.tile_pool(name="smallps", bufs=3, space="PSUM") as smallps,
            tc.tile_pool(name="oprojps", bufs=2, space="PSUM") as oprojps,
            tc.tile_pool(name="dram", bufs=1, space="DRAM") as dram,
        ):
            wq_sb = consts.tile([128, G, NDB, H], BF16)
            wk_sb = consts.tile([128, NDB, H], BF16)
            wv_sb = consts.tile([128, NDB, H], BF16)
            wo_sb = consts.tile([128, G, D], BF16)
            mask_sb = consts.tile([128, 128], BF16)
            ident = consts.tile([128, 128], BF16)

            kT = persist.tile([128, T], BF16)
            v_ext = persist.tile([128, NTB, VEXT_STRIDE], BF16)

            rs_in = [dram.tile([n, D], BF16, tag=f"rsw{ch}", name=f"rsw{ch}")
                     for ch, (s, n) in enumerate(RS_CHUNKS)]
            rs_out = [dram.tile([n // N_CORES, D], BF16, tag=f"rso{ch}",
                                name=f"rso{ch}")
                      for ch, (s, n) in enumerate(RS_CHUNKS)]

            # ---- initial constant DMAs (queue-balanced by need time:
            # sync: x q0a,q1,q3; scalar: wk,wq g0,g1; gpsimd: mask,cs0,
            # x q0b, wv, x q2, wq g2, g3) ----
            nc.gpsimd.dma_start(out=mask_sb[:], in_=mask_ext[:])
            nc.scalar.dma_start(out=wk_sb[:], in_=wk_ext[:])
            make_identity(nc, ident[:])
            nc.vector.memset(v_ext[:, :, 128:129], 1.0)

            # mutable emission state
            state = {
                "xq": {},      # (panel, quarter) -> sbuf tile
                "cs": {},      # panel -> (cos, sin) sbuf tiles
                "qT": {},      # (panel, g) -> roped qT tile [128, 512]
                "pts": {},     # (g, sb) -> (tile, col0) P^T tiles of cur panel
                "oT": {},      # g -> oT tile [128, 512] of cur att panel
            }

            def fetch_x(p, split_first=False):
                for q in range(NQ):
                    xq = xqp.tile([128, QDB, TP_SIZE], BF16, tag="xq",
                                  name=f"xq{p}_{q}")
                    if split_first and q == 0:
                        h = QDB // 2
                        nc.sync.dma_start(
                            out=xq[:, 0:h, :],
                            in_=xt_ext[:, p, 0:h, :])
                        nc.gpsimd.dma_start(
                            out=xq[:, h:QDB, :],
                            in_=xt_ext[:, p, h:QDB, :])
                    else:
                        nc.sync.dma_start(
                            out=xq[:],
                            in_=xt_ext[:, p, q * QDB:(q + 1) * QDB, :])
                    state["xq"][(p, q)] = xq

            def fetch_cs(p):
                tsl = slice(p * TP_SIZE, (p + 1) * TP_SIZE)
                cos_sb = csp.tile([H, TP_SIZE], BF16, tag="cos",
                                  name=f"cos{p}")
                sin_sb = csp.tile([H, TP_SIZE], BF16, tag="sin",
                                  name=f"sin{p}")
                nc.gpsimd.dma_start(out=cos_sb[:], in_=cos_ext[:, tsl])
                nc.gpsimd.dma_start(out=sin_sb[:], in_=sin_ext[:, tsl])
                state["cs"][p] = (cos_sb, sin_sb)

            def rope(p, raw, dst):
                """dst = raw*cos + halfswap(raw)*sin for panel p [128,512]."""
                cos_sb, sin_sb = state["cs"][p]
                sw = ropep.tile([128, TP_SIZE], BF16, tag="ropesw",
                                name=f"sw{p}")
                t1 = ropep.tile([128, TP_SIZE], BF16, tag="ropet1",
                                name=f"t1{p}")
                nc.scalar.dma_start(out=sw[0:64, :], in_=raw[64:128, :])
                nc.scalar.dma_start(out=sw[64:128, :], in_=raw[0:64, :])
                nc.vector.tensor_tensor(out=t1[:], in0=raw[:], in1=cos_sb[:],
                                        op=mybir.AluOpType.mult)
                nc.vector.tensor_tensor(out=sw[:], in0=sw[:], in1=sin_sb[:],
                                        op=mybir.AluOpType.mult)
                nc.vector.tensor_tensor(out=dst[:], in0=t1[:], in1=sw[:],
                                        op=mybir.AluOpType.add)

            def qkv_unit(p, which):
                """One QKV output for panel p: 'k' | 'v' | 0..G-1."""
                ps = qkv_ps.tile([128, TP_SIZE], F32, tag="qkv",
                                 name=f"qkv{p}_{which}")
                if which == "k":
                    w = wk_sb
                elif which == "v":
                    w = wv_sb
                else:
                    w = wq_sb[:, which]
                for db in range(NDB):
                    xq = state["xq"][(p, db // QDB)]
                    nc.tensor.matmul(
                        ps[:], w[:, db, :], xq[:, db % QDB, :],
                        start=(db == 0), stop=(db == NDB - 1))
                tsl = slice(p * TP_SIZE, (p + 1) * TP_SIZE)
                if which == "k":
                    nc.scalar.copy(kT[:, tsl], ps[:])
                    rope(p, kT[:, tsl], kT[:, tsl])
                elif which == "v":
                    vraw = scp.tile([128, TP_SIZE], BF16, tag="vraw",
                                    bufs=1, name=f"vraw{p}")
                    nc.scalar.copy(vraw[:], ps[:])
                    for j in range(4):
                        sb = 4 * p + j
                        pst = smallps.tile([128, 128], BF16, tag="sm",
                                           name=f"vtr{p}_{j}")
                        nc.tensor.transpose(
                            pst[:], vraw[:, j * 128:(j + 1) * 128], ident[:])
                        nc.vector.tensor_copy(v_ext[:, sb, 0:128], pst[:])
                else:
                    g = which
                    qt = qtp.tile([128, TP_SIZE], BF16, tag=f"qT{g}",
                                  name=f"qT{p}_{g}")
                    nc.vector.tensor_copy(qt[:], ps[:])
                    rope(p, qt[:], qt[:])
                    state["qT"][(p, g)] = qt

            def scores_unit(p, g):
                """S^T + exp for panel p head g -> pts tiles."""
                qt = state["qT"][(p, g)]
                for sb in range(4 * p + 4):
                    jj = sb - 4 * p
                    c0 = max(jj, 0) * 128   # first needed col (local)
                    w = TP_SIZE - c0
                    ps_s = sps.tile([128, TP_SIZE], F32, tag="s",
                                    name=f"s{p}_{g}_{sb}")
                    nc.tensor.matmul(
                        ps_s[:, 0:w], kT[:, sb * 128:(sb + 1) * 128],
                        qt[:, c0:TP_SIZE], start=True, stop=True)
                    pt = ptp.tile([128, TP_SIZE], BF16, tag="pt",
                                  name=f"pt{p}_{g}_{sb}")
                    nc.scalar.activation(pt[:, 0:w], ps_s[:, 0:w], EXP,
                                         scale=SCALE)
                    if jj >= 0:
                        nc.vector.tensor_tensor(
                            out=pt[:, 0:128], in0=pt[:, 0:128], in1=mask_sb[:],
                            op=mybir.AluOpType.mult)
                    state["pts"][(p, g, sb)] = (pt, c0)

            def pv_mm(p, g, j):
                """PV matmuls + DVE normalize for t-block j of head g."""
                if g not in state["oT"] or state["oT"][g][1] != p:
                    oT = scp.tile([128, TP_SIZE], BF16, tag=f"oT{g}",
                                  bufs=1, name=f"oT{p}_{g}")
                    state["oT"][g] = (oT, p)
                tb = 4 * p + j
                ps_pv = smallps.tile([128, 132], F32, tag="sm",
                                     name=f"pv{p}_{g}_{j}")
                for sb in range(tb + 1):
                    # local col of t-block j within this pts tile
                    pt, c0 = state["pts"][(p, g, sb)]
                    lo = j * 128 - c0
                    nc.tensor.matmul(
                        ps_pv[:, 0:129], pt[:, lo:lo + 128],
                        v_ext[:, sb, 0:129],
                        start=(sb == 0), stop=(sb == tb),
                        skip_group_check=True)
                rc = scp.tile([128, 1], F32, tag="rc", bufs=4,
                              name=f"rc{p}_{g}_{j}")
                nc.vector.reciprocal(rc[:], ps_pv[:, 128:129])
                ob = scp.tile([128, 128], BF16, tag="ob",
                              bufs=4, name=f"ob{p}_{g}_{j}")
                nc.vector.tensor_scalar_mul(ob[:], ps_pv[:, 0:128], rc[:])
                return ob

            def pv_tr(p, g, j, ob):
                """PE transpose of normalized block + DVE copy into oT."""
                oT = state["oT"][g][0]
                ps_tr = smallps.tile([128, 128], BF16, tag="sm",
                                     name=f"tr{p}_{g}_{j}")
                nc.tensor.transpose(ps_tr[:], ob[:], ident[:])
                nc.vector.tensor_copy(oT[:, j * 128:(j + 1) * 128],
                                      ps_tr[:])

            def pv_unit(p, g, js):
                """Software-pipelined PV/transpose over t-blocks js."""
                obs = []
                for idx, j in enumerate(js):
                    obs.append((j, pv_mm(p, g, j)))
                    if idx >= 1:
                        jq, obq = obs.pop(0)
                        pv_tr(p, g, jq, obq)
                for jq, obq in obs:
                    pv_tr(p, g, jq, obq)

            def oproj_unit(p, j):
                """o_proj for t-block j of panel p + its ReduceScatter."""
                tb = 4 * p + j
                ch = next(i for i, (s, n) in enumerate(RS_CHUNKS)
                          if s <= tb * 128 < s + n)
                row = tb * 128 - RS_CHUNKS[ch][0]
                last = p == NTP - 1
                for dq in range(4):
                    osb = osbp.tile([128, D // 4], BF16, tag="osb",
                                    name=f"osb{tb}_{dq}")
                    for dp in range(2):
                        od = oprojps.tile([128, 512], F32, tag="od",
                                          name=f"od{tb}_{dq}_{dp}")
                        dc = dq * 2 + dp
                        for g in range(G):
                            nc.tensor.matmul(
                                od[:],
                                state["oT"][g][0][:, j * 128:(j + 1) * 128],
                                wo_sb[:, g, dc * 512:(dc + 1) * 512],
                                start=(g == 0), stop=(g == G - 1),
                                skip_group_check=True)
                        eng = 1 if last else dq % 2
                        if eng == 0:
                            nc.scalar.copy(
                                osb[:, dp * 512:(dp + 1) * 512], od[:])
                        else:
                            nc.vector.tensor_copy(
                                osb[:, dp * 512:(dp + 1) * 512], od[:])
                    q = nc.sync if last else nc.scalar
                    q.dma_start(
                        out=rs_in[ch][row:row + 128,
                                      dq * 1024:(dq + 1) * 1024],
                        in_=osb[:])
                state.setdefault("rs_done", {}).setdefault(ch, 0)
                state["rs_done"][ch] += 1
                if state["rs_done"][ch] == RS_CHUNKS[ch][1] // 128:
                    nc.gpsimd.collective_compute(
                        "ReduceScatter",
                        mybir.AluOpType.add,
                        replica_groups=[list(range(N_CORES))],
                        ins=[rs_in[ch].opt()],
                        outs=[rs_out[ch].opt()],
                    )
                    s, n = RS_CHUNKS[ch]
                    nc.gpsimd.dma_start(
                        out=out_ext[s // N_CORES:
                                    s // N_CORES + n // N_CORES, :],
                        in_=rs_out[ch][:])

            # ---- block 0: QKV panel 0, DMA-arrival-ordered ----
            # All 6 outputs accumulate simultaneously (borrowing the idle
            # scores/oproj/small PSUM banks); (output, quarter) pairs are
            # emitted in the order their weight/x DMAs land across the 3
            # queues, so the PE streams instead of waiting for any queue.
            fetch_cs(0)
            xq0 = {}
            for q in range(NQ):
                xq0[q] = xqp.tile([128, QDB, TP_SIZE], BF16, tag="xq",
                                  name=f"xq0_{q}")
                state["xq"][(0, q)] = xq0[q]
            h = QDB // 2
            nc.sync.dma_start(out=xq0[0][:, 0:h, :], in_=xt_ext[:, 0, 0:h, :])
            nc.gpsimd.dma_start(out=xq0[0][:, h:QDB, :],
                                in_=xt_ext[:, 0, h:QDB, :])
            nc.sync.dma_start(out=xq0[1][:], in_=xt_ext[:, 0, QDB:2 * QDB, :])
            nc.scalar.dma_start(out=wq_sb[:, 0], in_=wq_ext[:, 0])
            nc.gpsimd.dma_start(out=wv_sb[:], in_=wv_ext[:])
            nc.sync.dma_start(out=xq0[3][:],
                              in_=xt_ext[:, 0, 3 * QDB:4 * QDB, :])
            nc.gpsimd.dma_start(out=xq0[2][:],
                                in_=xt_ext[:, 0, 2 * QDB:3 * QDB, :])
            nc.scalar.dma_start(out=wq_sb[:, 1], in_=wq_ext[:, 1])
            nc.gpsimd.dma_start(out=wq_sb[:, 2], in_=wq_ext[:, 2])
            nc.gpsimd.dma_start(out=wq_sb[:, 3], in_=wq_ext[:, 3])
            outs0 = ["k", "v", 0, 1, 2, 3]
            pools0 = [sps, sps, qkv_ps, smallps, oprojps, oprojps]
            tags0 = ["s", "s", "qkv", "sm", "od", "od"]
            ps0 = {}
            for o, pool, tg in zip(outs0, pools0, tags0):
                ps0[o] = pool.tile([128, TP_SIZE], F32, tag=tg,
                                   name=f"qkv0_{o}")
            PAIR_ORDER = [("k", 0), ("k", 1), ("v", 0), ("v", 1),
                          (0, 0), (0, 1), ("k", 3), ("v", 3),
                          ("k", 2), ("v", 2), (0, 2), (0, 3),
                          (1, 0), (1, 1), (1, 2), (1, 3),
                          (2, 0), (2, 1), (2, 2), (2, 3),
                          (3, 0), (3, 1), (3, 2), (3, 3)]
            done = {o: 0 for o in outs0}
            for pi, (o, q) in enumerate(PAIR_ORDER):
                w = (wk_sb if o == "k" else
                     wv_sb if o == "v" else wq_sb[:, o])
                for i in range(QDB):
                    db = q * QDB + i
                    nc.tensor.matmul(
                        ps0[o][:], w[:, db, :],
                        state["xq"][(0, q)][:, i, :],
                        start=(done[o] == 0),
                        stop=(done[o] == NDB - 1),
                        skip_group_check=True)
                    done[o] += 1
                if pi == 5:
                    fetch_x(1)
                if pi == 7:
                    fetch_cs(1)
                    nc.gpsimd.dma_start(out=wo_sb[:], in_=wo_ext[:])
            # copy-outs, ropes, v transpose for panel 0 (all PSUM
            # copy-outs first: vtr transposes reuse the "sm" slots)
            nc.scalar.copy(kT[:, 0:TP_SIZE], ps0["k"][:])
            rope(0, kT[:, 0:TP_SIZE], kT[:, 0:TP_SIZE])
            vraw = scp.tile([128, TP_SIZE], BF16, tag="vraw", bufs=1,
                            name="vraw0")
            nc.scalar.copy(vraw[:], ps0["v"][:])
            for g in range(G):
                qt = qtp.tile([128, TP_SIZE], BF16, tag=f"qT{g}",
                              name=f"qT0_{g}")
                nc.scalar.copy(qt[:], ps0[g][:])
                rope(0, qt[:], qt[:])
                state["qT"][(0, g)] = qt
            for j in range(4):
                pst = smallps.tile([128, 128], BF16, tag="sm",
                                   name=f"vtr0_{j}")
                nc.tensor.transpose(
                    pst[:], vraw[:, j * 128:(j + 1) * 128], ident[:])
                nc.vector.tensor_copy(v_ext[:, j, 0:128], pst[:])

            # ---- blocks 1..4: att(b-1) interleaved with QKV(b) ----
            # t-blocks processed j=3..0 so the output rows that gate the
            # final ReduceScatter chunks complete as early as possible;
            # the g3 tail staggers PV(j) / transpose(j) / oproj(j) so the
            # PE never sits on the DVE normalize turnaround.
            JSEQ = [1, 2, 3, 0]
            for b in range(1, NTP + 1):
                qkv = ([lambda w=w, b=b: qkv_unit(b, w)
                        for w in ["k", "v", 0, 1, 2, 3]]
                       if b < NTP else [])
                p = b - 1
                att = [lambda p=p: scores_unit(p, 0)]
                for g in range(G - 1):
                    def pv_sc(g=g, p=p):
                        obs = [(j, pv_mm(p, g, j)) for j in JSEQ]
                        scores_unit(p, g + 1)
                        for j, ob in obs:
                            pv_tr(p, g, j, ob)
                    att.append(pv_sc)
                # g3 tail: stagger PV/tr/oproj across units
                tail_state = {"obs": []}

                def pv3_push(j, p=p):
                    tail_state["obs"].append((j, pv_mm(p, G - 1, j)))

                def tr_pop(p=p):
                    j, ob = tail_state["obs"].pop(0)
                    pv_tr(p, G - 1, j, ob)
                    return j

                att.append(lambda: pv3_push(JSEQ[0]))
                att.append(lambda: (pv3_push(JSEQ[1]), tr_pop()))
                att.append(lambda p=p: (pv3_push(JSEQ[2]), tr_pop(),
                                        oproj_unit(p, JSEQ[0])))
                att.append(lambda p=p: (pv3_push(JSEQ[3]), tr_pop(),
                                        oproj_unit(p, JSEQ[1])))
                att.append(lambda p=p: (tr_pop(),
                                        oproj_unit(p, JSEQ[2])))
                att.append(lambda p=p: oproj_unit(p, JSEQ[3]))
                # interleave: att[0], qkv[0], att[1], qkv[1], ...
                n_u = max(len(att), len(qkv))
                for i in range(n_u):
                    if i < len(att):
                        att[i]()
                    if i < len(qkv):
                        qkv[i]()
                    if i == 0 and b + 1 < NTP:
                        fetch_cs(b + 1)
                    if i == 2 and b + 1 < NTP:
                        fetch_x(b + 1)

    nc.compile()
    return nc


def get_nc():
    if "nc" not in _NC_CACHE:
        _NC_CACHE["nc"] = _build_nc()
    return _NC_CACHE["nc"]


def make_in_maps(x, positions, w_q, w_k, w_v, w_o):
    """Host-side sharding + RoPE table / mask precompute."""
    x = np.ascontiguousarray(np.asarray(x, np.float32))
    positions = np.asarray(positions)

    half = H // 2
    inv_freq = 1.0 / (THETA ** (np.arange(half, dtype=np.float32) / half))
    ang = positions.astype(np.float32)[:, None] * inv_freq[None, :]  # [T, 64]
    cos = np.cos(ang)   # [T, 64]
    sin = np.sin(ang)
    cos_t = np.empty((H, T), np.float32)
    sin_t = np.empty((H, T), np.float32)
    cos_t[0:half] = cos.T
    cos_t[half:] = cos.T
    sin_t[0:half] = -sin.T
    sin_t[half:] = sin.T
    cos_t = cos_t.astype(ml_dtypes.bfloat16)
    sin_t = sin_t.astype(ml_dtypes.bfloat16)

    # mask[s, t] = 1 if s <= t (lower-left of P^T allowed region)
    idx = np.arange(128)
    maskp = (idx[:, None] <= idx[None, :]).astype(ml_dtypes.bfloat16)

    xt = x.astype(ml_dtypes.bfloat16).T  # [D, T]
    xt4 = np.ascontiguousarray(
        xt.reshape(NDB, 128, NTP, TP_SIZE).transpose(1, 2, 0, 3))
    w_q = np.asarray(w_q, np.float32).reshape(D, NH, H).astype(
        ml_dtypes.bfloat16)
    w_k = np.asarray(w_k, np.float32).reshape(D, KH, H).astype(
        ml_dtypes.bfloat16)
    w_v = np.asarray(w_v, np.float32).reshape(D, KH, H).astype(
        ml_dtypes.bfloat16)
    w_o = np.asarray(w_o, np.float32).reshape(NH, H, D).astype(
        ml_dtypes.bfloat16)

    def blk(w):
        """[D, n] -> [128, NDB, n] with row d = a*128 + p."""
        return np.ascontiguousarray(
            w.reshape(NDB, 128, -1).transpose(1, 0, 2))

    in_maps = []
    for c in range(N_CORES):
        # wq g-major: [128, G, NDB, H] so each head's chunk is contiguous
        wq_c = w_q[:, G * c:G * (c + 1), :]            # [D, G, H]
        wq_blk = np.ascontiguousarray(
            wq_c.reshape(NDB, 128, G, H).transpose(1, 2, 0, 3))
        in_maps.append({
            "xt": xt4,
            "wq": wq_blk,
            "wk": blk(w_k[:, c, :]),
            "wv": blk(w_v[:, c, :]),
            "wo": np.ascontiguousarray(
                w_o[G * c:G * (c + 1)].reshape(G, 128, D)
                .transpose(1, 0, 2)),
            "cos_t": cos_t,
            "sin_t": sin_t,
            "maskp": maskp,
        })
    return in_maps


def assemble_output(results):
    """results: list of 8 per-core dicts with 'out' [T//8, D] bf16."""
    out = np.empty((T, D), np.float32)
    for c in range(N_CORES):
        o = np.asarray(results[c]["out"], np.float32)
        for s, n in RS_CHUNKS:
            k = n // N_CORES
            out[s + c * k:s + (c + 1) * k] = o[s // N_CORES:s // N_CORES + k]
    return out


def kernel(x, positions, w_q, w_k, w_v, w_o):
    from concourse.bass_utils import run_bass_kernel_spmd

    _enable_ldw_opt()
    nc = get_nc()
    in_maps = make_in_maps(x, positions, w_q, w_k, w_v, w_o)
    res = run_bass_kernel_spmd(nc, in_maps, core_ids=list(range(N_CORES)))
    return assemble_output(res.results)

